# revision 11
# baseline (speedup 1.0000x reference)
"""KAN layer (cubic B-spline, 9 basis fns) as a single fused matmul on 8 trn2 cores.

Math: out[b,o] = sum_{i,r} coeff[o,i,r] * B_r(x[b,i]) + bias[o], x ~ U[0,1).

On x in [0,1) the spline space restricted to knot spans [0,1/3),[1/3,2/3),[2/3,1)
is the 6-dim space of C^2 piecewise cubics with breaks {1/3, 2/3}, spanned by
  phi = [1, x, (x-1/2)^2, (x-1/2)^3, (x-1/3)_+^3, (x-2/3)_+^3]
Each B_r == T[r,:] . phi exactly.  Folding T into the coefficients turns the
whole layer into one K=1280 matmul:
  out[b,o] = sum_{j=1..5, i} G[o,i,j] * phi_j(x[b,i]) + bias_eff[o]
with G = coeff . T and bias_eff = bias + sum_i G[:,i,0].

Sharding: data-parallel on batch (4096 rows/core), weights replicated.

v2 (vs baseline): fp16 x/weights/features/out (PE streams 1 row/cycle either
way, but fp16 halves DMA + SBUF traffic and weight-load time, and errs ~0.2%
vs the 2% gate); DMA issues spread across engine queues and chunked so the
first matmul starts ~6us earlier; ACT Square table preloaded under the DMA
wait; vector work rebalanced ACT/DVE/Pool so no engine exceeds the PE window.
"""

import os
import sys

import numpy as np

sys.path.insert(0, "/opt/trn_rl_repo")

import concourse.bass as bass
import concourse.mybir as mybir
import concourse.tile as tile
from concourse import bacc
from concourse.bass_utils import run_bass_kernel_spmd

F32 = mybir.dt.float32
F16 = mybir.dt.float16
AF = mybir.ActivationFunctionType
ALU = mybir.AluOpType

N_CORES = 8
B_FULL = 32768
IN_DIM = 256
OUT_DIM = 256
N_BASIS = 9
BC = B_FULL // N_CORES  # 4096 batch rows per core
P = 128
KC = 0.5  # centering point for the polynomial features
KA, KB = 1.0 / 3.0, 2.0 / 3.0  # interior knots inside [0,1)
N_FEAT = 5
N_KCHUNK = N_FEAT * IN_DIM // P  # 10
MM_N = 512  # matmul moving free dim

# exposed for test.py: last BassKernelResults (exec_time_ns when BASS_TRACE=1)
LAST_RESULT = None
_PROGRAM_CACHE = {}


def _bspline_basis_f64(x, t, degree=3):
    xe = x[..., None]
    b = ((xe >= t[:-1]) & (xe < t[1:])).astype(x.dtype)
    last_span = (t[:-1] < t[1:]) & (t[1:] >= t[-1])
    b = np.where((xe >= t[-1]) & last_span, 1.0, b)
    for d in range(1, degree + 1):
        d1 = t[d:-1] - t[: -d - 1]
        d2 = t[d + 1 :] - t[1:-d]
        s1 = np.where(d1 > 0, d1, 1.0)
        s2 = np.where(d2 > 0, d2, 1.0)
        w1 = np.where(d1 > 0, (xe - t[: -d - 1]) / s1, 0.0)
        w2 = np.where(d2 > 0, (t[d + 1 :] - xe) / s2, 0.0)
        b = w1 * b[..., :-1] + w2 * b[..., 1:]
    return b


def _basis_to_power_T():
    """T (9,6): B_r(x) = sum_j T[r,j] phi_j(x) on [0,1), exact (fit res ~1e-15)."""
    internal = np.linspace(-1.0, 1.0, 7)[1:-1]
    knots = np.concatenate([np.full(4, -1.0), internal, np.full(4, 1.0)])
    xs = np.linspace(0.0, 1.0, 12001)[:-1]
    u = np.maximum(xs - KA, 0.0)
    v = np.maximum(xs - KB, 0.0)
    phi = np.stack(
        [np.ones_like(xs), xs, (xs - KC) ** 2, (xs - KC) ** 3, u**3, v**3], axis=-1
    )
    bv = _bspline_basis_f64(xs, knots)
    T, _, _, _ = np.linalg.lstsq(phi, bv, rcond=None)
    return T.T  # (9, 6)


def _build_program(bc=BC, l_chunk=1024):
    key = (bc, l_chunk)
    if key in _PROGRAM_CACHE:
        return _PROGRAM_CACHE[key]

    nc = bacc.Bacc()
    xt = nc.dram_tensor("xt", (2, P, bc), F16, kind="ExternalInput")
    w = nc.dram_tensor("w", (P, N_KCHUNK, OUT_DIM), F16, kind="ExternalInput")
    beff = nc.dram_tensor("beff", (P, 2), F32, kind="ExternalInput")
    out_t = nc.dram_tensor("outT", (2, P, bc), F16, kind="ExternalOutput")

    # first and last chunks are small: the first so the opening DMA is tiny
    # (matmuls start ~4us earlier), the last so the closing evict+DMA tail
    # is short.
    chunks = [(0, 512), (512, 1024), (1536, 1024), (2560, 1024), (3584, 512)]
    assert sum(sz for _, sz in chunks) == bc
    n_sc = len(chunks)

    with tile.TileContext(nc) as tc:
        with (
            tc.tile_pool(name="consts", bufs=1) as consts,
            tc.tile_pool(name="xp", bufs=3) as xp,
            tc.tile_pool(name="fp", bufs=3) as fp,
            tc.tile_pool(name="sp", bufs=2) as sp,
            tc.tile_pool(name="op", bufs=4) as op,
            tc.tile_pool(name="pp", bufs=4, space="PSUM") as pp,
        ):
            # --- prologue: input DMAs issued in parallel on distinct engine
            # queues so descriptor generation doesn't serialize, and x for
            # the small first chunk lands ASAP (its 2 K-chunks feed the
            # first matmuls directly, no vector work needed). w is split by
            # output half: the first 2 psum groups only need w0.
            w0_sb = consts.tile([P, N_KCHUNK, P], F16)
            w1_sb = consts.tile([P, N_KCHUNK, P], F16)
            b_sb = consts.tile([P, 2], F32)
            x_tiles = [[None] * 2 for _ in range(n_sc)]
            s0, z0 = chunks[0]
            for ic in range(2):
                x_tiles[0][ic] = xp.tile(
                    [P, z0], F16, tag=f"x{ic}_{z0}", name=f"xt0_{ic}"
                )
            nc.sync.dma_start(x_tiles[0][0], xt[0, :, s0 : s0 + z0])
            nc.gpsimd.dma_start(x_tiles[0][1], xt[1, :, s0 : s0 + z0])
            nc.scalar.dma_start(w0_sb, w[:, :, 0:P])
            nc.scalar.dma_start(w1_sb, w[:, :, P : 2 * P])
            nc.gpsimd.dma_start(b_sb, beff[:, :])

            nkc_sb = consts.tile([P, 1], F32)
            nc.vector.memset(nkc_sb, -KC)

            # Preload the ACT Square table during the DMA wait so the first
            # real Square doesn't eat the ~1.3us ACT_TABLE_LOAD on the
            # critical path.
            warm = consts.tile([P, 1], F32)
            nc.scalar.activation(warm, nkc_sb, AF.Square)
            # Ramp the PE p-state during the DMA wait: ~3us of continuous
            # dummy matmuls brings the tensor clock to max before the real
            # stream starts.
            warm16 = consts.tile([P, 64], F16)
            nc.vector.memset(warm16, 0.0)
            ps_warm = pp.tile([64, 64], F32, name="ps_warm")
            for _ in range(40):
                nc.tensor.matmul(ps_warm, warm16, warm16, start=True, stop=True)

            dmaq = [nc.sync, nc.gpsimd]
            for sc in range(n_sc):
                # prefetch next chunk's x on spare queues
                if sc + 1 < n_sc:
                    s_n, z_n = chunks[sc + 1]
                    for ic in range(2):
                        x_tiles[sc + 1][ic] = xp.tile(
                            [P, z_n], F16, tag=f"x{ic}_{z_n}", name=f"xt{sc + 1}_{ic}"
                        )
                        dmaq[ic].dma_start(
                            x_tiles[sc + 1][ic], xt[ic, :, s_n : s_n + z_n]
                        )

                s_c, z_c = chunks[sc]
                n_nb = z_c // MM_N
                w_sbs = [w0_sb, w1_sb]
                feats = []
                for ic in range(2):
                    x_t = x_tiles[sc][ic]
                    # all-fp16 chain: DVE tensor_scalar 2-scalar ops run at 4x
                    # and tensor_tensor at 2x when every operand is 2-byte;
                    # scalar_tensor_tensor never accelerates, so avoid it.
                    # c-centered: sq = (x-c)^2 [ACT], p3 = sq*(x-c) [DVE tt]
                    xc = sp.tile([P, z_c], F16, tag=f"xc{ic}_{z_c}")
                    nc.vector.tensor_scalar_add(xc, x_t, -KC)
                    sq = fp.tile([P, z_c], F16, tag=f"sq{ic}_{z_c}")
                    nc.scalar.activation(sq, x_t, AF.Square, bias=nkc_sb[:, :])
                    p3 = fp.tile([P, z_c], F16, tag=f"p3{ic}_{z_c}")
                    nc.vector.tensor_tensor(p3, sq, xc, ALU.mult)
                    # a-knot: ra = relu(x-a) [DVE ts 4x], u3 = ra^2*ra
                    ra = sp.tile([P, z_c], F16, tag=f"ra{ic}_{z_c}")
                    nc.vector.tensor_scalar(ra, x_t, -KA, 0.0, ALU.add, ALU.max)
                    sa = sp.tile([P, z_c], F16, tag=f"sa{ic}_{z_c}")
                    nc.scalar.activation(sa, ra, AF.Square)
                    u3 = fp.tile([P, z_c], F16, tag=f"u3{ic}_{z_c}")
                    nc.vector.tensor_tensor(u3, sa, ra, ALU.mult)
                    # b-knot: rb = relu(x-b), v3 = rb^2*rb
                    rb = sp.tile([P, z_c], F16, tag=f"rb{ic}_{z_c}")
                    nc.vector.tensor_scalar(rb, x_t, -KB, 0.0, ALU.add, ALU.max)
                    sb = sp.tile([P, z_c], F16, tag=f"sb{ic}_{z_c}")
                    nc.scalar.activation(sb, rb, AF.Square)
                    v3 = fp.tile([P, z_c], F16, tag=f"v3{ic}_{z_c}")
                    nc.vector.tensor_tensor(v3, sb, rb, ALU.mult)
                    feats.append([x_t, sq, p3, u3, v3])

                for nb in range(n_nb):
                    nsl = slice(nb * MM_N, (nb + 1) * MM_N)
                    for oc in range(2):
                        ps = pp.tile([P, MM_N], F32)
                        kidx = 0
                        for j in range(N_FEAT):
                            for ic in range(2):
                                nc.tensor.matmul(
                                    ps,
                                    w_sbs[oc][:, j * 2 + ic, :],
                                    feats[ic][j][:, nsl],
                                    start=(kidx == 0),
                                    stop=(kidx == 2 * N_FEAT - 1),
                                )
                                kidx += 1
                        o_sb = op.tile([P, MM_N], F16, tag="o")
                        # evictions alternate ACT / DVE so neither exceeds
                        # the PE window
                        if (nb * 2 + oc) % 2 == 0:
                            nc.scalar.activation(
                                o_sb, ps, AF.Identity, bias=b_sb[:, oc : oc + 1]
                            )
                        else:
                            nc.vector.tensor_scalar(
                                o_sb, ps, b_sb[:, oc : oc + 1], None, ALU.add
                            )
                        dmaq[(nb + oc) % 2].dma_start(
                            out_t[
                                oc,
                                :,
                                s_c + nb * MM_N : s_c + (nb + 1) * MM_N,
                            ],
                            o_sb,
                        )

    nc.finalize()
    _PROGRAM_CACHE[key] = nc
    return nc


def _prep_weights(coeff, bias):
    T = _basis_to_power_T()
    G = np.einsum("oir,rj->oij", coeff.astype(np.float64), T)
    bias_eff = (bias.astype(np.float64) + G[:, :, 0].sum(axis=1)).astype(np.float32)
    wk = G[:, :, 1:]  # (o, i, 5)
    w_lhs_t = np.transpose(wk, (2, 1, 0)).reshape(N_FEAT * IN_DIM, OUT_DIM)
    w_host = np.ascontiguousarray(
        w_lhs_t.reshape(N_KCHUNK, P, OUT_DIM).transpose(1, 0, 2)
    ).astype(np.float16)  # (128, 10, 256): [p, kchunk, o]
    beff_host = np.ascontiguousarray(bias_eff.reshape(2, P).T)  # (128, 2)
    return w_host, beff_host


def kernel(x, coeff, bias):
    global LAST_RESULT
    x = np.asarray(x, dtype=np.float32)
    coeff = np.asarray(coeff, dtype=np.float32)
    bias = np.asarray(bias, dtype=np.float32)
    assert x.shape == (B_FULL, IN_DIM)
    assert coeff.shape == (OUT_DIM, IN_DIM, N_BASIS)

    w_host, beff_host = _prep_weights(coeff, bias)

    in_maps = []
    for c in range(N_CORES):
        xs = x[c * BC : (c + 1) * BC, :]  # (4096, 256)
        xt = np.ascontiguousarray(xs.T).reshape(2, P, BC).astype(np.float16)
        in_maps.append({"xt": xt, "w": w_host, "beff": beff_host})

    nc = _build_program()
    res = run_bass_kernel_spmd(nc, in_maps, core_ids=list(range(N_CORES)))
    LAST_RESULT = res

    out = np.empty((B_FULL, OUT_DIM), dtype=np.float32)
    for c in range(N_CORES):
        ot = res.results[c]["outT"].astype(np.float32).reshape(OUT_DIM, BC)
        out[c * BC : (c + 1) * BC, :] = ot.T
    return out


# revision 16
# speedup vs baseline: 1.1330x; 1.1330x over previous
"""KAN layer (cubic B-spline, 9 basis fns) as a single fused matmul on 8 trn2 cores.

Math: out[b,o] = sum_{i,r} coeff[o,i,r] * B_r(x[b,i]) + bias[o], x ~ U[0,1).

On x in [0,1) the spline space restricted to knot spans [0,1/3),[1/3,2/3),[2/3,1)
is the 6-dim space of C^2 piecewise cubics with breaks {1/3, 2/3}, spanned by
  phi = [1, x, (x-1/2)^2, (x-1/2)^3, (x-1/3)_+^3, (x-2/3)_+^3]
Each B_r == T[r,:] . phi exactly.  Folding T into the coefficients turns the
whole layer into one K=1280 matmul:
  out[b,o] = sum_{j=1..5, i} G[o,i,j] * phi_j(x[b,i]) + bias_eff[o]
with G = coeff . T and bias_eff = bias + sum_i G[:,i,0].

Sharding: data-parallel on batch (4096 rows/core), weights replicated.

v2 (vs baseline): fp16 x/weights/features/out (PE streams 1 row/cycle either
way, but fp16 halves DMA + SBUF traffic and weight-load time, and errs ~0.2%
vs the 2% gate); DMA issues spread across engine queues and chunked so the
first matmul starts ~6us earlier; ACT Square table preloaded under the DMA
wait; vector work rebalanced ACT/DVE/Pool so no engine exceeds the PE window.
"""

import os
import sys

import numpy as np

sys.path.insert(0, "/opt/trn_rl_repo")

import concourse.bass as bass
import concourse.mybir as mybir
import concourse.tile as tile
from concourse import bacc
from concourse.bass_utils import run_bass_kernel_spmd

F32 = mybir.dt.float32
F16 = mybir.dt.float16
AF = mybir.ActivationFunctionType
ALU = mybir.AluOpType

N_CORES = 8
B_FULL = 32768
IN_DIM = 256
OUT_DIM = 256
N_BASIS = 9
BC = B_FULL // N_CORES  # 4096 batch rows per core
P = 128
KC = 0.5  # centering point for the polynomial features
KA, KB = 1.0 / 3.0, 2.0 / 3.0  # interior knots inside [0,1)
N_FEAT = 5
N_KCHUNK = N_FEAT * IN_DIM // P  # 10
MM_N = 512  # matmul moving free dim

# exposed for test.py: last BassKernelResults (exec_time_ns when BASS_TRACE=1)
LAST_RESULT = None
_PROGRAM_CACHE = {}


def _bspline_basis_f64(x, t, degree=3):
    xe = x[..., None]
    b = ((xe >= t[:-1]) & (xe < t[1:])).astype(x.dtype)
    last_span = (t[:-1] < t[1:]) & (t[1:] >= t[-1])
    b = np.where((xe >= t[-1]) & last_span, 1.0, b)
    for d in range(1, degree + 1):
        d1 = t[d:-1] - t[: -d - 1]
        d2 = t[d + 1 :] - t[1:-d]
        s1 = np.where(d1 > 0, d1, 1.0)
        s2 = np.where(d2 > 0, d2, 1.0)
        w1 = np.where(d1 > 0, (xe - t[: -d - 1]) / s1, 0.0)
        w2 = np.where(d2 > 0, (t[d + 1 :] - xe) / s2, 0.0)
        b = w1 * b[..., :-1] + w2 * b[..., 1:]
    return b


def _basis_to_power_T():
    """T (9,6): B_r(x) = sum_j T[r,j] phi_j(x) on [0,1), exact (fit res ~1e-15)."""
    internal = np.linspace(-1.0, 1.0, 7)[1:-1]
    knots = np.concatenate([np.full(4, -1.0), internal, np.full(4, 1.0)])
    xs = np.linspace(0.0, 1.0, 12001)[:-1]
    u = np.maximum(xs - KA, 0.0)
    v = np.maximum(xs - KB, 0.0)
    phi = np.stack(
        [np.ones_like(xs), xs, (xs - KC) ** 2, (xs - KC) ** 3, u**3, v**3], axis=-1
    )
    bv = _bspline_basis_f64(xs, knots)
    T, _, _, _ = np.linalg.lstsq(phi, bv, rcond=None)
    return T.T  # (9, 6)


def _build_program(bc=BC, l_chunk=1024):
    key = (bc, l_chunk)
    if key in _PROGRAM_CACHE:
        return _PROGRAM_CACHE[key]

    nc = bacc.Bacc()
    xt = nc.dram_tensor("xt", (2, P, bc), F16, kind="ExternalInput")
    w = nc.dram_tensor("w", (P, N_KCHUNK, OUT_DIM), F16, kind="ExternalInput")
    beff = nc.dram_tensor("beff", (P, 2), F32, kind="ExternalInput")
    out_t = nc.dram_tensor("outT", (2, P, bc), F16, kind="ExternalOutput")

    # small chunks at both ends: the first two so the opening DMAs are tiny
    # (matmuls start ~5us earlier while the input stream ramps), the last
    # two so the closing evict+DMA tail is short.
    sizes = [512, 512, 1024, 1024, 512, 512]
    starts = [sum(sizes[:i]) for i in range(len(sizes))]
    chunks = list(zip(starts, sizes))
    assert sum(sz for _, sz in chunks) == bc
    n_sc = len(chunks)

    with tile.TileContext(nc) as tc:
        with (
            tc.tile_pool(name="consts", bufs=1) as consts,
            tc.tile_pool(name="xp", bufs=4) as xp,
            tc.tile_pool(name="fp", bufs=3) as fp,
            tc.tile_pool(name="sp", bufs=2) as sp,
            tc.tile_pool(name="op", bufs=4) as op,
            tc.tile_pool(name="pp", bufs=4, space="PSUM") as pp,
        ):
            # --- prologue: input DMAs issued up front, spread across the
            # three DMA-capable queues (sync/gpsimd/scalar) so the input
            # stream uses ~3 rings in parallel (a single ring only sustains
            # ~50-90 GB/s). The j=0 weight K-chunks get their own tiny DMA
            # so the very first matmuls (which consume x directly) can
            # start after ~64KB of weights.
            w0a_sb = consts.tile([P, 2, P], F16)
            w0b_sb = consts.tile([P, N_KCHUNK - 2, P], F16)
            w1_sb = consts.tile([P, N_KCHUNK, P], F16)
            b_sb = consts.tile([P, 2], F32)
            nc.scalar.dma_start(w0a_sb, w[:, 0:2, 0:P])
            x_tiles = [[None] * 2 for _ in range(n_sc)]
            for sc in range(n_sc):
                s_n, z_n = chunks[sc]
                for ic in range(2):
                    x_tiles[sc][ic] = xp.tile(
                        [P, z_n], F16, tag=f"x{ic}_{z_n}", name=f"xt{sc}_{ic}"
                    )
            nc.sync.dma_start(x_tiles[0][0], xt[0, :, 0:512])
            nc.gpsimd.dma_start(x_tiles[0][1], xt[1, :, 0:512])
            nc.scalar.dma_start(w0b_sb, w[:, 2:N_KCHUNK, 0:P])
            nc.scalar.dma_start(w1_sb, w[:, :, P : 2 * P])
            rr = 0
            dmaq3 = [nc.sync, nc.gpsimd, nc.scalar]
            for sc in range(1, n_sc):
                s_n, z_n = chunks[sc]
                for ic in range(2):
                    dmaq3[rr % 3].dma_start(
                        x_tiles[sc][ic], xt[ic, :, s_n : s_n + z_n]
                    )
                    rr += 1
            nc.gpsimd.dma_start(b_sb, beff[:, :])

            nkc_sb = consts.tile([P, 1], F32)
            nc.vector.memset(nkc_sb, -KC)

            # Preload the ACT Square table during the DMA wait so the first
            # real Square doesn't eat the ~1.3us ACT_TABLE_LOAD on the
            # critical path.
            warm = consts.tile([P, 1], F32)
            nc.scalar.activation(warm, nkc_sb, AF.Square)
            # Ramp the PE p-state during the DMA wait: ~2us of continuous
            # dummy matmuls brings the tensor clock up before the real
            # stream starts.
            warm16 = consts.tile([P, 64], F16)
            nc.vector.memset(warm16, 0.0)
            ps_warm = pp.tile([64, 64], F32, name="ps_warm")
            for _ in range(16):
                nc.tensor.matmul(ps_warm, warm16, warm16, start=True, stop=True)

            dmaq = [nc.sync, nc.gpsimd]
            for sc in range(n_sc):

                s_c, z_c = chunks[sc]
                n_nb = z_c // MM_N
                feats = []
                for ic in range(2):
                    x_t = x_tiles[sc][ic]
                    # all-fp16 chain: DVE tensor_scalar 2-scalar ops run at 4x
                    # and tensor_tensor at 2x when every operand is 2-byte;
                    # scalar_tensor_tensor never accelerates, so avoid it.
                    # c-centered: sq = (x-c)^2 [ACT], p3 = sq*(x-c) [DVE tt]
                    xc = sp.tile([P, z_c], F16, tag=f"xc{ic}_{z_c}")
                    nc.vector.tensor_scalar_add(xc, x_t, -KC)
                    sq = fp.tile([P, z_c], F16, tag=f"sq{ic}_{z_c}")
                    nc.scalar.activation(sq, x_t, AF.Square, bias=nkc_sb[:, :])
                    p3 = fp.tile([P, z_c], F16, tag=f"p3{ic}_{z_c}")
                    nc.vector.tensor_tensor(p3, sq, xc, ALU.mult)
                    # a-knot: ra = relu(x-a) [DVE ts 4x], u3 = ra^2*ra
                    ra = sp.tile([P, z_c], F16, tag=f"ra{ic}_{z_c}")
                    nc.vector.tensor_scalar(ra, x_t, -KA, 0.0, ALU.add, ALU.max)
                    sa = sp.tile([P, z_c], F16, tag=f"sa{ic}_{z_c}")
                    nc.scalar.activation(sa, ra, AF.Square)
                    u3 = fp.tile([P, z_c], F16, tag=f"u3{ic}_{z_c}")
                    nc.vector.tensor_tensor(u3, sa, ra, ALU.mult)
                    # b-knot: rb = relu(x-b), v3 = rb^2*rb
                    rb = sp.tile([P, z_c], F16, tag=f"rb{ic}_{z_c}")
                    nc.vector.tensor_scalar(rb, x_t, -KB, 0.0, ALU.add, ALU.max)
                    sb = sp.tile([P, z_c], F16, tag=f"sb{ic}_{z_c}")
                    nc.scalar.activation(sb, rb, AF.Square)
                    v3 = fp.tile([P, z_c], F16, tag=f"v3{ic}_{z_c}")
                    nc.vector.tensor_tensor(v3, sb, rb, ALU.mult)
                    feats.append([x_t, sq, p3, u3, v3])

                for nb in range(n_nb):
                    nsl = slice(nb * MM_N, (nb + 1) * MM_N)
                    for oc in range(2):
                        ps = pp.tile([P, MM_N], F32)
                        kidx = 0
                        for j in range(N_FEAT):
                            for ic in range(2):
                                if oc == 1:
                                    w_ap = w1_sb[:, j * 2 + ic, :]
                                elif j == 0:
                                    w_ap = w0a_sb[:, ic, :]
                                else:
                                    w_ap = w0b_sb[:, (j - 1) * 2 + ic, :]
                                nc.tensor.matmul(
                                    ps,
                                    w_ap,
                                    feats[ic][j][:, nsl],
                                    start=(kidx == 0),
                                    stop=(kidx == 2 * N_FEAT - 1),
                                )
                                kidx += 1
                        o_sb = op.tile([P, MM_N], F16, tag="o")
                        # evictions alternate ACT / DVE so neither exceeds
                        # the PE window
                        if (nb * 2 + oc) % 2 == 0:
                            nc.scalar.activation(
                                o_sb, ps, AF.Identity, bias=b_sb[:, oc : oc + 1]
                            )
                        else:
                            nc.vector.tensor_scalar(
                                o_sb, ps, b_sb[:, oc : oc + 1], None, ALU.add
                            )
                        dmaq[(nb + oc) % 2].dma_start(
                            out_t[
                                oc,
                                :,
                                s_c + nb * MM_N : s_c + (nb + 1) * MM_N,
                            ],
                            o_sb,
                        )

    nc.finalize()
    _PROGRAM_CACHE[key] = nc
    return nc


def _prep_weights(coeff, bias):
    T = _basis_to_power_T()
    G = np.einsum("oir,rj->oij", coeff.astype(np.float64), T)
    bias_eff = (bias.astype(np.float64) + G[:, :, 0].sum(axis=1)).astype(np.float32)
    wk = G[:, :, 1:]  # (o, i, 5)
    w_lhs_t = np.transpose(wk, (2, 1, 0)).reshape(N_FEAT * IN_DIM, OUT_DIM)
    w_host = np.ascontiguousarray(
        w_lhs_t.reshape(N_KCHUNK, P, OUT_DIM).transpose(1, 0, 2)
    ).astype(np.float16)  # (128, 10, 256): [p, kchunk, o]
    beff_host = np.ascontiguousarray(bias_eff.reshape(2, P).T)  # (128, 2)
    return w_host, beff_host


def kernel(x, coeff, bias):
    global LAST_RESULT
    x = np.asarray(x, dtype=np.float32)
    coeff = np.asarray(coeff, dtype=np.float32)
    bias = np.asarray(bias, dtype=np.float32)
    assert x.shape == (B_FULL, IN_DIM)
    assert coeff.shape == (OUT_DIM, IN_DIM, N_BASIS)

    w_host, beff_host = _prep_weights(coeff, bias)

    in_maps = []
    for c in range(N_CORES):
        xs = x[c * BC : (c + 1) * BC, :]  # (4096, 256)
        xt = np.ascontiguousarray(xs.T).reshape(2, P, BC).astype(np.float16)
        in_maps.append({"xt": xt, "w": w_host, "beff": beff_host})

    nc = _build_program()
    res = run_bass_kernel_spmd(nc, in_maps, core_ids=list(range(N_CORES)))
    LAST_RESULT = res

    out = np.empty((B_FULL, OUT_DIM), dtype=np.float32)
    for c in range(N_CORES):
        ot = res.results[c]["outT"].astype(np.float32).reshape(OUT_DIM, BC)
        out[c * BC : (c + 1) * BC, :] = ot.T
    return out


# revision 19
# speedup vs baseline: 1.1532x; 1.0178x over previous
"""KAN layer (cubic B-spline, 9 basis fns) as a single fused matmul on 8 trn2 cores.

Math: out[b,o] = sum_{i,r} coeff[o,i,r] * B_r(x[b,i]) + bias[o], x ~ U[0,1).

On x in [0,1) the spline space restricted to knot spans [0,1/3),[1/3,2/3),[2/3,1)
is the 6-dim space of C^2 piecewise cubics with breaks {1/3, 2/3}, spanned by
  phi = [1, x, (x-1/2)^2, (x-1/2)^3, (x-1/3)_+^3, (x-2/3)_+^3]
Each B_r == T[r,:] . phi exactly.  Folding T into the coefficients turns the
whole layer into one K=1280 matmul:
  out[b,o] = sum_{j=1..5, i} G[o,i,j] * phi_j(x[b,i]) + bias_eff[o]
with G = coeff . T and bias_eff = bias + sum_i G[:,i,0].

Sharding: data-parallel on batch (4096 rows/core), weights replicated.

v2 (vs baseline): fp16 x/weights/features/out (PE streams 1 row/cycle either
way, but fp16 halves DMA + SBUF traffic and weight-load time, and errs ~0.2%
vs the 2% gate); DMA issues spread across engine queues and chunked so the
first matmul starts ~6us earlier; ACT Square table preloaded under the DMA
wait; vector work rebalanced ACT/DVE/Pool so no engine exceeds the PE window.
"""

import os
import sys

import numpy as np

sys.path.insert(0, "/opt/trn_rl_repo")

import concourse.bass as bass
import concourse.mybir as mybir
import concourse.tile as tile
from concourse import bacc
from concourse.bass_utils import run_bass_kernel_spmd

F32 = mybir.dt.float32
F16 = mybir.dt.float16
AF = mybir.ActivationFunctionType
ALU = mybir.AluOpType

N_CORES = 8
B_FULL = 32768
IN_DIM = 256
OUT_DIM = 256
N_BASIS = 9
BC = B_FULL // N_CORES  # 4096 batch rows per core
P = 128
KC = 0.5  # centering point for the polynomial features
KA, KB = 1.0 / 3.0, 2.0 / 3.0  # interior knots inside [0,1)
N_FEAT = 5
N_KCHUNK = N_FEAT * IN_DIM // P  # 10
MM_N = 512  # matmul moving free dim

# exposed for test.py: last BassKernelResults (exec_time_ns when BASS_TRACE=1)
LAST_RESULT = None
_PROGRAM_CACHE = {}


def _bspline_basis_f64(x, t, degree=3):
    xe = x[..., None]
    b = ((xe >= t[:-1]) & (xe < t[1:])).astype(x.dtype)
    last_span = (t[:-1] < t[1:]) & (t[1:] >= t[-1])
    b = np.where((xe >= t[-1]) & last_span, 1.0, b)
    for d in range(1, degree + 1):
        d1 = t[d:-1] - t[: -d - 1]
        d2 = t[d + 1 :] - t[1:-d]
        s1 = np.where(d1 > 0, d1, 1.0)
        s2 = np.where(d2 > 0, d2, 1.0)
        w1 = np.where(d1 > 0, (xe - t[: -d - 1]) / s1, 0.0)
        w2 = np.where(d2 > 0, (t[d + 1 :] - xe) / s2, 0.0)
        b = w1 * b[..., :-1] + w2 * b[..., 1:]
    return b


def _basis_to_power_T():
    """T (9,6): B_r(x) = sum_j T[r,j] phi_j(x) on [0,1), exact (fit res ~1e-15)."""
    internal = np.linspace(-1.0, 1.0, 7)[1:-1]
    knots = np.concatenate([np.full(4, -1.0), internal, np.full(4, 1.0)])
    xs = np.linspace(0.0, 1.0, 12001)[:-1]
    u = np.maximum(xs - KA, 0.0)
    v = np.maximum(xs - KB, 0.0)
    phi = np.stack(
        [np.ones_like(xs), xs, (xs - KC) ** 2, (xs - KC) ** 3, u**3, v**3], axis=-1
    )
    bv = _bspline_basis_f64(xs, knots)
    T, _, _, _ = np.linalg.lstsq(phi, bv, rcond=None)
    return T.T  # (9, 6)


def _build_program(bc=BC, l_chunk=1024):
    key = (bc, l_chunk)
    if key in _PROGRAM_CACHE:
        return _PROGRAM_CACHE[key]

    nc = bacc.Bacc()
    xt = nc.dram_tensor("xt", (2, P, bc), F16, kind="ExternalInput")
    w = nc.dram_tensor("w", (P, N_KCHUNK, OUT_DIM), F16, kind="ExternalInput")
    beff = nc.dram_tensor("beff", (P, 2), F32, kind="ExternalInput")
    out_t = nc.dram_tensor("outT", (2, P, bc), F16, kind="ExternalOutput")

    # small chunks at both ends: the first so the opening DMAs are tiny
    # (matmuls start ~4us earlier), the last so the closing evict+DMA tail
    # is short.
    sizes = [512, 1024, 1024, 1024, 512]
    starts = [sum(sizes[:i]) for i in range(len(sizes))]
    chunks = list(zip(starts, sizes))
    assert sum(sz for _, sz in chunks) == bc
    n_sc = len(chunks)

    with tile.TileContext(nc) as tc:
        with (
            tc.tile_pool(name="consts", bufs=1) as consts,
            tc.tile_pool(name="xp", bufs=4) as xp,
            tc.tile_pool(name="fp", bufs=3) as fp,
            tc.tile_pool(name="sp", bufs=2) as sp,
            tc.tile_pool(name="op", bufs=4) as op,
            tc.tile_pool(name="pp", bufs=4, space="PSUM") as pp,
        ):
            # --- prologue: input DMAs issued up front, spread across the
            # three DMA-capable queues (sync/gpsimd/scalar) so the input
            # stream uses ~3 rings in parallel (a single ring only sustains
            # ~50-90 GB/s). The j=0 weight K-chunks get their own tiny DMA
            # so the very first matmuls (which consume x directly) can
            # start after ~64KB of weights.
            w0a_sb = consts.tile([P, 2, P], F16)
            w0b_sb = consts.tile([P, N_KCHUNK - 2, P], F16)
            w1_sb = consts.tile([P, N_KCHUNK, P], F16)
            b_sb = consts.tile([P, 2], F32)
            nc.scalar.dma_start(w0a_sb, w[:, 0:2, 0:P])
            x_tiles = [[None] * 2 for _ in range(n_sc)]
            for sc in range(n_sc):
                s_n, z_n = chunks[sc]
                for ic in range(2):
                    x_tiles[sc][ic] = xp.tile(
                        [P, z_n], F16, tag=f"x{ic}_{z_n}", name=f"xt{sc}_{ic}"
                    )
            # dedicated ring per x half (need-order within each ring), all
            # weights on the scalar ring so x never queues behind them
            nc.sync.dma_start(x_tiles[0][0], xt[0, :, 0:512])
            nc.gpsimd.dma_start(x_tiles[0][1], xt[1, :, 0:512])
            nc.scalar.dma_start(w0b_sb, w[:, 2:N_KCHUNK, 0:P])
            nc.scalar.dma_start(w1_sb, w[:, :, P : 2 * P])
            for sc in range(1, n_sc):
                s_n, z_n = chunks[sc]
                nc.sync.dma_start(x_tiles[sc][0], xt[0, :, s_n : s_n + z_n])
                nc.gpsimd.dma_start(x_tiles[sc][1], xt[1, :, s_n : s_n + z_n])
            nc.gpsimd.dma_start(b_sb, beff[:, :])

            nkc_sb = consts.tile([P, 1], F32)
            nc.vector.memset(nkc_sb, -KC)

            # Preload the ACT Square table during the DMA wait so the first
            # real Square doesn't eat the ~1.3us ACT_TABLE_LOAD on the
            # critical path.
            warm = consts.tile([P, 1], F32)
            nc.scalar.activation(warm, nkc_sb, AF.Square)
            # Ramp the PE p-state during the DMA wait: ~2us of continuous
            # dummy matmuls brings the tensor clock up before the real
            # stream starts.
            warm16 = consts.tile([P, 64], F16)
            nc.vector.memset(warm16, 0.0)
            ps_warm = pp.tile([64, 64], F32, name="ps_warm")
            for _ in range(36):
                nc.tensor.matmul(ps_warm, warm16, warm16, start=True, stop=True)

            dmaq = [nc.sync, nc.gpsimd]
            for sc in range(n_sc):

                s_c, z_c = chunks[sc]
                n_nb = z_c // MM_N
                feats = []
                for ic in range(2):
                    x_t = x_tiles[sc][ic]
                    # all-fp16 chain: DVE tensor_scalar 2-scalar ops run at 4x
                    # and tensor_tensor at 2x when every operand is 2-byte;
                    # scalar_tensor_tensor never accelerates, so avoid it.
                    # c-centered: sq = (x-c)^2 [ACT], p3 = sq*(x-c) [DVE tt]
                    xc = sp.tile([P, z_c], F16, tag=f"xc{ic}_{z_c}")
                    nc.vector.tensor_scalar_add(xc, x_t, -KC)
                    sq = fp.tile([P, z_c], F16, tag=f"sq{ic}_{z_c}")
                    nc.scalar.activation(sq, x_t, AF.Square, bias=nkc_sb[:, :])
                    p3 = fp.tile([P, z_c], F16, tag=f"p3{ic}_{z_c}")
                    nc.vector.tensor_tensor(p3, sq, xc, ALU.mult)
                    # a-knot: ra = relu(x-a) [DVE ts 4x], u3 = ra^2*ra
                    ra = sp.tile([P, z_c], F16, tag=f"ra{ic}_{z_c}")
                    nc.vector.tensor_scalar(ra, x_t, -KA, 0.0, ALU.add, ALU.max)
                    sa = sp.tile([P, z_c], F16, tag=f"sa{ic}_{z_c}")
                    nc.scalar.activation(sa, ra, AF.Square)
                    u3 = fp.tile([P, z_c], F16, tag=f"u3{ic}_{z_c}")
                    nc.vector.tensor_tensor(u3, sa, ra, ALU.mult)
                    # b-knot: rb = relu(x-b), v3 = rb^2*rb
                    rb = sp.tile([P, z_c], F16, tag=f"rb{ic}_{z_c}")
                    nc.vector.tensor_scalar(rb, x_t, -KB, 0.0, ALU.add, ALU.max)
                    sb = sp.tile([P, z_c], F16, tag=f"sb{ic}_{z_c}")
                    nc.scalar.activation(sb, rb, AF.Square)
                    v3 = fp.tile([P, z_c], F16, tag=f"v3{ic}_{z_c}")
                    nc.vector.tensor_tensor(v3, sb, rb, ALU.mult)
                    feats.append([x_t, sq, p3, u3, v3])

                for nb in range(n_nb):
                    nsl = slice(nb * MM_N, (nb + 1) * MM_N)
                    for oc in range(2):
                        ps = pp.tile([P, MM_N], F32)
                        kidx = 0
                        for j in range(N_FEAT):
                            for ic in range(2):
                                if oc == 1:
                                    w_ap = w1_sb[:, j * 2 + ic, :]
                                elif j == 0:
                                    w_ap = w0a_sb[:, ic, :]
                                else:
                                    w_ap = w0b_sb[:, (j - 1) * 2 + ic, :]
                                nc.tensor.matmul(
                                    ps,
                                    w_ap,
                                    feats[ic][j][:, nsl],
                                    start=(kidx == 0),
                                    stop=(kidx == 2 * N_FEAT - 1),
                                )
                                kidx += 1
                        o_sb = op.tile([P, MM_N], F16, tag="o")
                        # evictions alternate ACT / DVE so neither exceeds
                        # the PE window
                        if (nb * 2 + oc) % 2 == 0:
                            nc.scalar.activation(
                                o_sb, ps, AF.Identity, bias=b_sb[:, oc : oc + 1]
                            )
                        else:
                            nc.vector.tensor_scalar(
                                o_sb, ps, b_sb[:, oc : oc + 1], None, ALU.add
                            )
                        dmaq[(nb + oc) % 2].dma_start(
                            out_t[
                                oc,
                                :,
                                s_c + nb * MM_N : s_c + (nb + 1) * MM_N,
                            ],
                            o_sb,
                        )

    nc.finalize()
    _PROGRAM_CACHE[key] = nc
    return nc


def _prep_weights(coeff, bias):
    T = _basis_to_power_T()
    G = np.einsum("oir,rj->oij", coeff.astype(np.float64), T)
    bias_eff = (bias.astype(np.float64) + G[:, :, 0].sum(axis=1)).astype(np.float32)
    wk = G[:, :, 1:]  # (o, i, 5)
    w_lhs_t = np.transpose(wk, (2, 1, 0)).reshape(N_FEAT * IN_DIM, OUT_DIM)
    w_host = np.ascontiguousarray(
        w_lhs_t.reshape(N_KCHUNK, P, OUT_DIM).transpose(1, 0, 2)
    ).astype(np.float16)  # (128, 10, 256): [p, kchunk, o]
    beff_host = np.ascontiguousarray(bias_eff.reshape(2, P).T)  # (128, 2)
    return w_host, beff_host


def kernel(x, coeff, bias):
    global LAST_RESULT
    x = np.asarray(x, dtype=np.float32)
    coeff = np.asarray(coeff, dtype=np.float32)
    bias = np.asarray(bias, dtype=np.float32)
    assert x.shape == (B_FULL, IN_DIM)
    assert coeff.shape == (OUT_DIM, IN_DIM, N_BASIS)

    w_host, beff_host = _prep_weights(coeff, bias)

    in_maps = []
    for c in range(N_CORES):
        xs = x[c * BC : (c + 1) * BC, :]  # (4096, 256)
        xt = np.ascontiguousarray(xs.T).reshape(2, P, BC).astype(np.float16)
        in_maps.append({"xt": xt, "w": w_host, "beff": beff_host})

    nc = _build_program()
    res = run_bass_kernel_spmd(nc, in_maps, core_ids=list(range(N_CORES)))
    LAST_RESULT = res

    out = np.empty((B_FULL, OUT_DIM), dtype=np.float32)
    for c in range(N_CORES):
        ot = res.results[c]["outT"].astype(np.float32).reshape(OUT_DIM, BC)
        out[c * BC : (c + 1) * BC, :] = ot.T
    return out
